# revision 1
# baseline (speedup 1.0000x reference)
"""Trainium2 Bass kernel for nn_MeanShift (retrieval_knn).

Full-input contract: kernel(**inputs) -> (loss, purity).

Strategy (8 NeuronCores):
  - Shard the memory bank (K=128000) across the 8 cores (16000 rows each),
    queries/targets replicated.
  - Host prep: L2-normalize bank rows (0.4% of total FLOPs), transpose to
    [C, K_local] layout per core so the matmul streams bank columns.
  - Device (per core): sim[b,k] = sum_c t[b,c]*bank_norm[k,c] via TensorE
    (PSUM accumulation over 4 chunks of C=512), ScalarE evicts PSUM->SBUF,
    VectorE max/max_index produce the top-8 (value, index) per 2000-wide
    k-chunk per row -> 64 candidates per row per core.
  - Host epilogue: reduce 8*64=512 candidates/row to the global top-5
    (matching jax.lax.top_k tie-breaking on fp32 distances), then compute
    dist_q at those 1280 indices + label purity.

Selection correctness: the global top-5 of each row is contained in the
union of per-chunk top-8s (8 >= 5 per any chunk), and per-row ordering by
raw sim (unnormalized t) equals ordering by cosine distance since the
per-row scale 1/||t_b|| > 0.
"""

import numpy as np
import ml_dtypes

import jax
from jax.experimental.shard_map import shard_map
from jax.sharding import Mesh, PartitionSpec

import concourse.bass as bass
import concourse.bacc as bacc
import concourse.mybir as mybir
import concourse.tile as tile
from concourse import bass2jax

N_CORES = 8
B = 256          # batch (rows of query/current_target)
C = 512          # feature dim
K = 128000       # memory bank size
KL = K // N_CORES  # 16000 bank rows per core
KT = 500         # matmul k-tile width (PSUM bank holds 512 fp32)
GRP = 4          # k-tiles per max-scan chunk (v2 path)
CHUNK = KT * GRP   # 2000 elements per DVE max8 scan (v2 path)
N_GRP = KL // CHUNK  # 8 scan chunks per core (v2 path)
NCAND = 8 * N_GRP    # 64 candidates per row per core (v2 path)
TOPK = 5
EPS = 1e-12


def groups_for(kl):
    """v1 scan-chunk widths. Six 500-wide leading groups cut the DVE
    start-up ramp; 1000-wide steady-state chunks schedule tighter than
    2000 (TimelineSim: 84.5us vs 87.8us per core for kl=16000)."""
    if kl >= 4000 and (kl - 3000) % 1000 == 0:
        return [500] * 6 + [1000] * ((kl - 3000) // 1000)
    assert kl % KT == 0
    return [KT] * (kl // KT)

# bfloat16 halves DMA + PE time; fp32 is the accuracy-safe fallback.
# Validated on the fixed inputs: bf16 changes 15/256 rows' top-5 with min
# 5th/6th sim gap 2.9e-4 (>> HW accumulation noise), loss rel err 4.8e-5,
# purity identical (0.0) -- well inside the 2e-2 gate.
DTYPE = mybir.dt.bfloat16

# v2 (tagged single-scan) constants. Device computes sims scaled to
# |sim| <= 0.25 (host passes t_norm/4; actual |sim| ~ 0.05). Per 500-wide
# matmul tile the PE appends three rank-1 accumulations, in order:
#   +4.0   -- rounds sim onto the 2^-21 grid (exponent pinned at 2^2)
#   -4.0   -- Sterbenz-exact unshift, psum = q(sim), a 2^-21 multiple
#   +id*2^-25, id in [0,16) the 125-wide subchunk of the column -- exact
#          (ulp <= 2^-26 for |q| < 0.25), and SUB-quantum, so packed
#          ordering matches q(sim) ordering to within one quantum.
# One max8 scan returns packed = q(sim) + id*2^-25; the host decodes
# id = (packed/2^-25) mod 16 (q/2^-25 is a multiple of 16 for the
# positive sims that matter) and re-derives exact values by recomputing
# the winners' 125-wide windows.
N_SUB_PER_KT = 4          # 4 subchunks of 125 per 500-wide k-tile
SUB = KT // N_SUB_PER_KT  # 125
N_SUB = CHUNK // SUB      # 16 subchunk ids per 2000-wide scan chunk
TAG_EPS = 2.0 ** -25
QCONST = 4.0
SIM_SCALE = 0.25          # host scales t_norm by this before casting

LAST_RESULTS = None    # per-core output dicts of the most recent run


def build_nc(dtype=DTYPE, kl=KL, with_index=True):
    """Build the single-core Bass program (SPMD across 8 cores).

    with_index=False (v3): drop the max_index pass and cand_i output --
    the host recovers indices by recomputing the <=8 winning 500-wide
    chunks per row (candidate slot -> chunk is static). Halves DVE work.
    """
    groups = [KT] * (kl // KT) if not with_index else groups_for(kl)
    n_grp = len(groups)
    ncand = 8 * n_grp
    mx = max(groups)
    # Bacc (not raw Bass): its compile() passes split multi-semaphore waits
    # (move_matmul_waits_to_ldweights / generate_event_semaphores) that the
    # walrus codegen's 1-wait-per-instruction limit requires.
    nc = bacc.Bacc()
    bankT = nc.declare_dram_parameter("bankT", [C, kl], dtype, isOutput=False)
    tT = nc.declare_dram_parameter("tT", [C, B], dtype, isOutput=False)
    cand_v = nc.declare_dram_parameter(
        "cand_v", [B, ncand], mybir.dt.float32, isOutput=True
    )
    cand_i = None
    if with_index:
        cand_i = nc.declare_dram_parameter(
            "cand_i", [B, ncand], mybir.dt.uint32, isOutput=True
        )

    bankT_r = bankT.rearrange("(c p) k -> p c k", p=128)  # [128, 4, kl]
    tT_r = tT.rearrange("(c p) b -> p c b", p=128)        # [128, 4, B]

    with tile.TileContext(nc) as tc:
        with (
            tc.tile_pool(name="const", bufs=1) as constp,
            # bufs=4: with the max_index pass gone the PE chain paces the
            # schedule, and 4-deep bank prefetch keeps it fed (model:
            # 67.5us vs 70.5us at bufs=3; saturates at 4).
            tc.tile_pool(name="bank", bufs=4) as bankp,
            tc.tile_pool(name="sim", bufs=2) as simp,
            tc.tile_pool(name="cand", bufs=1) as candp,
            tc.tile_pool(name="ps", bufs=8, space="PSUM") as psp,
        ):
            tw = constp.tile([128, 4, B], dtype)
            nc.sync.dma_start(tw[:], tT_r[:])

            vals = [
                candp.tile([128, n_grp, 8], mybir.dt.float32, tag=f"v{b}", name=f"vals{b}")
                for b in range(2)
            ]
            idxs = None
            if with_index:
                idxs = [
                    candp.tile([128, n_grp, 8], mybir.dt.uint32, tag=f"i{b}", name=f"idxs{b}")
                    for b in range(2)
                ]

            kt = 0
            for g, chunk in enumerate(groups):
                sims = [
                    simp.tile([128, mx], mybir.dt.float32, tag=f"s{b}", name=f"sim{b}")
                    for b in range(2)
                ]
                for j in range(chunk // KT):
                    bk = bankp.tile([128, 4, KT], dtype, tag="bank")
                    if kt == 0:
                        # split the first load per c-chunk so the first
                        # matmul starts after 1/4 of the transfer
                        # (model: 64.7us vs 67.5us)
                        for c in range(4):
                            nc.sync.dma_start(
                                bk[:, c, :], bankT_r[:, c, 0:KT]
                            )
                    else:
                        nc.sync.dma_start(
                            bk[:], bankT_r[:, :, kt * KT:(kt + 1) * KT]
                        )
                    for b in range(2):
                        ps = psp.tile([128, KT], mybir.dt.float32, tag="ps")
                        for c in range(4):
                            nc.tensor.matmul(
                                ps[:],
                                tw[:, c, b * 128:(b + 1) * 128],
                                bk[:, c, :],
                                start=(c == 0),
                                stop=(c == 3),
                            )
                        nc.scalar.copy(sims[b][:, j * KT:(j + 1) * KT], ps[:])
                    kt += 1
                for b in range(2):
                    nc.vector.max(vals[b][:, g, :], sims[b][:, 0:chunk])
                    if with_index:
                        nc.vector.max_index(
                            idxs[b][:, g, :], vals[b][:, g, :], sims[b][:, 0:chunk]
                        )

            for b in range(2):
                nc.sync.dma_start(cand_v[b * 128:(b + 1) * 128, :], vals[b][:])
                if with_index:
                    nc.sync.dma_start(cand_i[b * 128:(b + 1) * 128, :], idxs[b][:])

    return nc


def _make_consts():
    """Host-side constant rows for the v2 tag matmuls, bf16 [1, 3500].

    Layout: [0:128) ones (rank-1 stationary); [500:1000) +4.0;
    [1000:1500) -4.0; [1500+j*500 : 2000+j*500) tag row for kt%4 == j:
    id*2^-25 with id = ((j*500+n) // SUB) % 16. All exact in bf16.
    """
    c = np.zeros((1, 3500), np.float32)
    c[0, 0:128] = 1.0
    c[0, 500:1000] = QCONST
    c[0, 1000:1500] = -QCONST
    n = np.arange(KT)
    for j in range(4):
        ids = (j * KT + n) // SUB % N_SUB
        c[0, 1500 + j * 500:2000 + j * 500] = ids * TAG_EPS
    return c.astype(ml_dtypes.bfloat16)


def build_nc_v2(dtype=mybir.dt.bfloat16, kl=KL):
    """Tagged single-scan variant: one DVE max8 pass, no max_index."""
    assert dtype == mybir.dt.bfloat16
    n_grp = kl // CHUNK
    ncand = 8 * n_grp
    nc = bacc.Bacc()
    bankT = nc.declare_dram_parameter("bankT", [C, kl], dtype, isOutput=False)
    tT = nc.declare_dram_parameter("tT", [C, B], dtype, isOutput=False)
    consts = nc.declare_dram_parameter("consts", [1, 3500], dtype, isOutput=False)
    cand_v = nc.declare_dram_parameter(
        "cand_v", [B, ncand], mybir.dt.float32, isOutput=True
    )

    bankT_r = bankT.rearrange("(c p) k -> p c k", p=128)  # [128, 4, kl]
    tT_r = tT.rearrange("(c p) b -> p c b", p=128)        # [128, 4, B]

    with tile.TileContext(nc) as tc:
        with (
            tc.tile_pool(name="const", bufs=1) as constp,
            tc.tile_pool(name="bank", bufs=3) as bankp,
            tc.tile_pool(name="sim", bufs=2) as simp,
            tc.tile_pool(name="cand", bufs=1) as candp,
            tc.tile_pool(name="ps", bufs=8, space="PSUM") as psp,
        ):
            tw = constp.tile([128, 4, B], dtype)
            nc.sync.dma_start(tw[:], tT_r[:])
            cst = constp.tile([1, 3500], dtype)
            nc.sync.dma_start(cst[:], consts[:])
            ones_r = cst[0:1, 0:128]
            q_r = cst[0:1, 500:1000]
            nq_r = cst[0:1, 1000:1500]
            tag_r = [cst[0:1, 1500 + j * 500:2000 + j * 500] for j in range(4)]

            vals = [
                candp.tile([128, n_grp, 8], mybir.dt.float32,
                           tag=f"v{b}", name=f"vals{b}")
                for b in range(2)
            ]

            for g in range(n_grp):
                sims = [
                    simp.tile([128, CHUNK], mybir.dt.float32,
                              tag=f"s{b}", name=f"sim{b}")
                    for b in range(2)
                ]
                for j in range(GRP):
                    kt = g * GRP + j
                    bk = bankp.tile([128, 4, KT], dtype, tag="bank")
                    nc.sync.dma_start(
                        bk[:], bankT_r[:, :, kt * KT:(kt + 1) * KT]
                    )
                    for b in range(2):
                        ps = psp.tile([128, KT], mybir.dt.float32, tag="ps",
                                      name="ps")
                        for c in range(4):
                            nc.tensor.matmul(
                                ps[:],
                                tw[:, c, b * 128:(b + 1) * 128],
                                bk[:, c, :],
                                start=(c == 0), stop=False,
                            )
                        # quantize then tag: +4, -4, +id*2^-25 (in order)
                        nc.tensor.matmul(ps[:], ones_r, q_r,
                                         start=False, stop=False)
                        nc.tensor.matmul(ps[:], ones_r, nq_r,
                                         start=False, stop=False)
                        nc.tensor.matmul(ps[:], ones_r, tag_r[j % 4],
                                         start=False, stop=True)
                        nc.scalar.copy(sims[b][:, j * KT:(j + 1) * KT], ps[:])
                for b in range(2):
                    nc.vector.max(vals[b][:, g, :], sims[b][:])

            for b in range(2):
                nc.sync.dma_start(cand_v[b * 128:(b + 1) * 128, :], vals[b][:])

    return nc


# "v1": two DVE scans per chunk (max8 + max_index) -- simplest, and the
#       faster schedule under the TRN2 instruction cost model (87.8us vs
#       109.6us predicted per core; DVE-bound).
# "v2": tagged single-scan -- one DVE max8 pass; the PE quantizes sims
#       in-PSUM (+4/-4 rank-1s) and adds a sub-quantum subchunk tag that
#       the host decodes, trading DVE time for PE time. Better if real
#       silicon streams bf16 matmuls near the documented 131ns/MM rate.
# "v3": v1's matmul+max8 pipeline with NO max_index pass at all -- the
#       candidate slot already identifies the 500-wide chunk, so the host
#       recomputes the <=8 best chunks per row (~1 GFLOP) to recover exact
#       indices. Halves DVE work; model-predicted 70.5us vs 84.5us (v1).
# All validated on the fixed inputs (HW): v1 loss rel err 4.9e-5,
# v2 5.3e-6, v3 4.9e-5; purity exact in all.
MODE = "v3"

_NC_CACHE = {}


def _get_nc():
    key = (MODE, DTYPE)
    if key not in _NC_CACHE:
        if MODE == "v2":
            nc = build_nc_v2()
        elif MODE == "v3":
            nc = build_nc(DTYPE, with_index=False)
        else:
            nc = build_nc(DTYPE)
        nc.finalize()
        _NC_CACHE[key] = nc
    return _NC_CACHE[key]


class _SpmdExec:
    """Cached jitted shard_map over the bass_exec custom call.

    Mirrors bass2jax.run_bass_via_pjrt's multi-core path but builds the
    jitted executable once, so repeated calls skip retrace/recompile.
    """

    def __init__(self, nc):
        bass2jax.install_neuronx_cc_hook()
        part_name = (
            nc.partition_id_tensor.name if nc.partition_id_tensor else None
        )
        in_names, out_names, out_avals = [], [], []
        for alloc in nc.m.functions[0].allocations:
            if not isinstance(alloc, mybir.MemoryLocationSet):
                continue
            name = alloc.memorylocations[0].name
            if alloc.kind == "ExternalInput":
                if name != part_name:
                    in_names.append(name)
            elif alloc.kind == "ExternalOutput":
                out_names.append(name)
                out_avals.append(
                    jax.core.ShapedArray(
                        tuple(alloc.tensor_shape), mybir.dt.np(alloc.dtype)
                    )
                )
        self.in_names = list(in_names)
        self.out_names = out_names
        self.out_avals = out_avals
        n_params = len(in_names)
        n_outs = len(out_names)
        bind_names = in_names + out_names
        if part_name is not None:
            bind_names = bind_names + [part_name]
        bind_names = tuple(bind_names)

        def _body(*args):
            operands = list(args)
            if part_name is not None:
                operands.append(bass2jax.partition_id_tensor())
            outs = bass2jax._bass_exec_p.bind(
                *operands,
                out_avals=tuple(out_avals),
                in_names=bind_names,
                out_names=tuple(out_names),
                lowering_input_output_aliases=(),
                sim_require_finite=True,
                sim_require_nnan=True,
                nc=nc,
            )
            return tuple(outs)

        devices = jax.devices()[:N_CORES]
        self.mesh = Mesh(np.asarray(devices), ("core",))
        in_specs = (PartitionSpec("core"),) * (n_params + n_outs)
        out_specs = (PartitionSpec("core"),) * n_outs
        self.fn = jax.jit(
            shard_map(
                _body,
                mesh=self.mesh,
                in_specs=in_specs,
                out_specs=out_specs,
                check_rep=False,
            ),
            donate_argnums=tuple(range(n_params, n_params + n_outs)),
            keep_unused=True,
        )

    def zero_outs(self):
        return [
            np.zeros((N_CORES * a.shape[0], *a.shape[1:]), a.dtype)
            for a in self.out_avals
        ]

    def __call__(self, concat_inputs):
        """concat_inputs: list matching in_names, each (N_CORES*dim0, ...)."""
        out_arrs = self.fn(*concat_inputs, *self.zero_outs())
        return [
            {
                name: np.asarray(out_arrs[i]).reshape(
                    N_CORES, *self.out_avals[i].shape
                )[c]
                for i, name in enumerate(self.out_names)
            }
            for c in range(N_CORES)
        ]


_EXEC_CACHE = {}


def _get_exec():
    key = (MODE, DTYPE)
    if key not in _EXEC_CACHE:
        _EXEC_CACHE[key] = _SpmdExec(_get_nc())
    return _EXEC_CACHE[key]


def _np_dtype(dtype):
    return ml_dtypes.bfloat16 if dtype == mybir.dt.bfloat16 else np.float32


def _run_v1(exe, bank_sh, t, tT):
    """max8 + max_index path: returns per-row global top-5 indices."""
    global LAST_RESULTS
    np_dt = _np_dtype(DTYPE)
    tT_c = tT.astype(np_dt)
    concat = {
        "bankT": bank_sh,
        "tT": np.concatenate([tT_c] * N_CORES, axis=0),
    }
    results = exe([concat[n] for n in exe.in_names])
    LAST_RESULTS = results

    vals = np.stack([r["cand_v"] for r in results], axis=1)
    idx_l = np.stack(
        [r["cand_i"].astype(np.int64) for r in results], axis=1
    )
    groups = groups_for(KL)
    gbase = np.concatenate([[0], np.cumsum(groups)[:-1]]).astype(np.int64)
    base = (
        np.arange(N_CORES, dtype=np.int64)[None, :, None] * KL
        + np.repeat(gbase, 8)[None, None, :]
    )
    gidx = (idx_l + base).reshape(B, -1)            # global indices
    vals = vals.reshape(B, -1)                      # raw sim_t

    # Emulate the reference's comparison domain: fp32 dist_t with per-row
    # 1/||t_b|| folded back in; ties break toward the lowest global index.
    inv_t = 1.0 / np.maximum(np.linalg.norm(t, axis=1), EPS)   # [B]
    dist32 = (2.0 - 2.0 * vals * inv_t[:, None]).astype(np.float32)
    top5 = np.empty((B, TOPK), np.int64)
    for b in range(B):
        order = np.lexsort((gidx[b], dist32[b]))
        top5[b] = gidx[b][order[:TOPK]]
    return top5


N_WINDOWS = 10  # per-row candidate windows recomputed exactly on the host


def _run_v2(exe, bank_sh, t, bank):
    """Tagged single-scan path: returns per-row global top-5 indices."""
    global LAST_RESULTS
    bf = ml_dtypes.bfloat16
    t_n = t / np.maximum(np.linalg.norm(t, axis=1, keepdims=True), EPS)
    tw = np.ascontiguousarray((t_n * SIM_SCALE).T).astype(bf)   # [C, B]
    consts = _make_consts()
    concat = {
        "bankT": bank_sh,
        "tT": np.concatenate([tw] * N_CORES, axis=0),
        "consts": np.concatenate([consts] * N_CORES, axis=0),
    }
    results = exe([concat[n] for n in exe.in_names])
    LAST_RESULTS = results

    # packed candidates [B, N_CORES, NCAND]
    packed = np.stack([r["cand_v"] for r in results], axis=1)
    pk = packed.reshape(B, -1).astype(np.float64)    # [B, 512]
    # packed = q(sim) + id*2^-25 with q a multiple of 2^-21 (positive sims)
    y = np.round(pk / TAG_EPS).astype(np.int64)      # exact integer
    dec_id = np.mod(y, N_SUB)
    qsim = pk - dec_id * TAG_EPS                     # quantized scaled sim
    # window start (global bank row) per candidate
    cores = np.repeat(np.arange(N_CORES, dtype=np.int64), NCAND)[None, :]
    groups = np.tile(
        np.repeat(np.arange(N_GRP, dtype=np.int64), 8), N_CORES
    )[None, :]
    wstart = cores * KL + groups * CHUNK + dec_id * SUB   # [B, 512]

    # top-N_WINDOWS candidates per row by qsim; recompute those 125-wide
    # windows exactly (fp32 over the bf16-cast operands, matching the
    # device's computation up to summation order) and take the exact top-5.
    order = np.argsort(-qsim, axis=1, kind="stable")[:, :N_WINDOWS]
    sel_start = np.take_along_axis(wstart, order, axis=1)     # [B, W]

    bank_bf = bank.astype(bf).astype(np.float32)              # [K, C]
    t_bf = (t_n * SIM_SCALE).astype(bf).astype(np.float32)    # [B, C]
    flat_idx = (sel_start[:, :, None] +
                np.arange(SUB, dtype=np.int64)[None, None, :])  # [B, W, SUB]
    rows = bank_bf[flat_idx.reshape(-1)].reshape(B, N_WINDOWS * SUB, C)
    wsims = np.einsum("bkc,bc->bk", rows, t_bf)               # [B, W*SUB]
    widx = flat_idx.reshape(B, -1)                            # [B, W*SUB]

    top5 = np.empty((B, TOPK), np.int64)
    for b in range(B):
        # windows may overlap -> dedupe indices, keep exact values
        o = np.lexsort((widx[b], -wsims[b]))
        seen, picks = set(), []
        for i in o:
            gi = widx[b, i]
            if gi in seen:
                continue
            seen.add(gi)
            picks.append(gi)
            if len(picks) == TOPK:
                break
        top5[b] = picks
    return top5


def _run_v3(exe, bank_sh, t, bank):
    """Index-free path: per-chunk top-8 values only (exact fp32, a
    deterministic superset of the per-chunk top-5); the host recovers
    indices by recomputing the <=8 best 500-wide chunks per row."""
    global LAST_RESULTS
    np_dt = _np_dtype(DTYPE)
    tT_c = np.ascontiguousarray(t.T).astype(np_dt)
    concat = {
        "bankT": bank_sh,
        "tT": np.concatenate([tT_c] * N_CORES, axis=0),
    }
    results = exe([concat[n] for n in exe.in_names])
    LAST_RESULTS = results

    n_grp = KL // KT                                 # 32 chunks of 500
    vals = np.stack([r["cand_v"] for r in results], axis=1)
    vals = vals.reshape(B, -1)                       # [B, 8*32*8=2048]
    # candidate slot -> global chunk start (chunk known from position)
    cores = np.repeat(np.arange(N_CORES, dtype=np.int64), 8 * n_grp)
    chunks = np.tile(np.repeat(np.arange(n_grp, dtype=np.int64), 8), N_CORES)
    wstart = (cores * KL + chunks * KT)[None, :]     # [1, 2048]

    # every true top-5 element is a candidate with a top-5 value, so the
    # top-8 candidate windows per row cover them deterministically
    order = np.argsort(-vals, axis=1, kind="stable")[:, :8]
    sel = np.take_along_axis(np.broadcast_to(wstart, vals.shape),
                             order, axis=1)          # [B, 8]

    bf = ml_dtypes.bfloat16
    bank_bf = bank.astype(bf).astype(np.float32)     # [K, C]
    t_bf = t.astype(bf).astype(np.float32)           # [B, C]
    top5 = np.empty((B, TOPK), np.int64)
    span = np.arange(KT, dtype=np.int64)
    for b in range(B):
        starts = np.unique(sel[b])
        widx = (starts[:, None] + span[None, :]).reshape(-1)
        wsims = bank_bf[widx] @ t_bf[b]              # exact bf16-input sims
        o = np.lexsort((widx, -wsims))
        top5[b] = widx[o[:TOPK]]
    return top5


def kernel(query, current_target, queue, labels, labels_queue):
    query = np.asarray(query, np.float32)
    t = np.asarray(current_target, np.float32)
    queue_f = np.asarray(queue, np.float32)
    labels = np.asarray(labels)
    labels_queue = np.asarray(labels_queue)

    # Host prep: normalize bank rows (fp32, matching reference), transpose.
    norms = np.maximum(np.linalg.norm(queue_f, axis=1), EPS)
    bank = queue_f / norms[:, None]                 # [K, C], normalized
    tT = np.ascontiguousarray(t.T)                  # [C, B]

    np_dt = _np_dtype(DTYPE)
    exe = _get_exec()
    # [8*C, KL]: core m's shard (rows m*C..(m+1)*C) is bank[m*KL:(m+1)*KL].T
    bank_sh = np.ascontiguousarray(
        bank.reshape(N_CORES, KL, C).transpose(0, 2, 1)
    ).astype(np_dt).reshape(N_CORES * C, KL)

    if MODE == "v2":
        top5 = _run_v2(exe, bank_sh, t, bank)
    elif MODE == "v3":
        top5 = _run_v3(exe, bank_sh, t, bank)
    else:
        top5 = _run_v1(exe, bank_sh, t, tT)

    # dist_q at the selected indices + purity.
    q_norm = query / np.maximum(
        np.linalg.norm(query, axis=1, keepdims=True), EPS
    )
    rows = bank[top5.reshape(-1)].reshape(B, TOPK, C)          # normalized
    nn_dist_q = 2.0 - 2.0 * np.einsum(
        "bjc,bc->bj", rows.astype(np.float64), q_norm.astype(np.float64)
    )
    loss = nn_dist_q.mean()
    matches = labels_queue[top5] == labels[:, None]
    purity = matches.mean()
    return (np.float32(loss), np.float32(purity))



# revision 9
# speedup vs baseline: 1356.9133x; 1356.9133x over previous
"""Trainium2 Bass kernel for nn_MeanShift (retrieval_knn).

Full-input contract: kernel(**inputs) -> (loss, purity).

Strategy (8 NeuronCores):
  - Shard the memory bank (K=128000) across the 8 cores (16000 rows each),
    queries/targets replicated.
  - Host prep: L2-normalize bank rows (0.4% of total FLOPs), transpose to
    [C, K_local] layout per core so the matmul streams bank columns.
  - Device (per core): sim[b,k] = sum_c t[b,c]*bank_norm[k,c] via TensorE
    (PSUM accumulation over 4 chunks of C=512), ScalarE evicts PSUM->SBUF,
    VectorE max/max_index produce the top-8 (value, index) per 2000-wide
    k-chunk per row -> 64 candidates per row per core.
  - Host epilogue: reduce 8*64=512 candidates/row to the global top-5
    (matching jax.lax.top_k tie-breaking on fp32 distances), then compute
    dist_q at those 1280 indices + label purity.

Selection correctness: the global top-5 of each row is contained in the
union of per-chunk top-8s (8 >= 5 per any chunk), and per-row ordering by
raw sim (unnormalized t) equals ordering by cosine distance since the
per-row scale 1/||t_b|| > 0.
"""

import numpy as np
import ml_dtypes

import jax
from jax.experimental.shard_map import shard_map
from jax.sharding import Mesh, PartitionSpec

import concourse.bass as bass
import concourse.bacc as bacc
import concourse.mybir as mybir
import concourse.tile as tile
from concourse import bass2jax

N_CORES = 8
B = 256          # batch (rows of query/current_target)
C = 512          # feature dim
K = 128000       # memory bank size
KL = K // N_CORES  # 16000 bank rows per core
KT = 500         # matmul k-tile width (PSUM bank holds 512 fp32)
GRP = 4          # k-tiles per max-scan chunk (v2 path)
CHUNK = KT * GRP   # 2000 elements per DVE max8 scan (v2 path)
N_GRP = KL // CHUNK  # 8 scan chunks per core (v2 path)
NCAND = 8 * N_GRP    # 64 candidates per row per core (v2 path)
TOPK = 5
EPS = 1e-12


def groups_for(kl):
    """v1 scan-chunk widths. Six 500-wide leading groups cut the DVE
    start-up ramp; 1000-wide steady-state chunks schedule tighter than
    2000 (TimelineSim: 84.5us vs 87.8us per core for kl=16000)."""
    if kl >= 4000 and (kl - 3000) % 1000 == 0:
        return [500] * 6 + [1000] * ((kl - 3000) // 1000)
    assert kl % KT == 0
    return [KT] * (kl // KT)

# bfloat16 halves DMA + PE time; fp32 is the accuracy-safe fallback.
# Validated on the fixed inputs: bf16 changes 15/256 rows' top-5 with min
# 5th/6th sim gap 2.9e-4 (>> HW accumulation noise), loss rel err 4.8e-5,
# purity identical (0.0) -- well inside the 2e-2 gate.
DTYPE = mybir.dt.bfloat16

# v2 (tagged single-scan) constants. Device computes sims scaled to
# |sim| <= 0.25 (host passes t_norm/4; actual |sim| ~ 0.05). Per 500-wide
# matmul tile the PE appends three rank-1 accumulations, in order:
#   +4.0   -- rounds sim onto the 2^-21 grid (exponent pinned at 2^2)
#   -4.0   -- Sterbenz-exact unshift, psum = q(sim), a 2^-21 multiple
#   +id*2^-25, id in [0,16) the 125-wide subchunk of the column -- exact
#          (ulp <= 2^-26 for |q| < 0.25), and SUB-quantum, so packed
#          ordering matches q(sim) ordering to within one quantum.
# One max8 scan returns packed = q(sim) + id*2^-25; the host decodes
# id = (packed/2^-25) mod 16 (q/2^-25 is a multiple of 16 for the
# positive sims that matter) and re-derives exact values by recomputing
# the winners' 125-wide windows.
N_SUB_PER_KT = 4          # 4 subchunks of 125 per 500-wide k-tile
SUB = KT // N_SUB_PER_KT  # 125
N_SUB = CHUNK // SUB      # 16 subchunk ids per 2000-wide scan chunk
TAG_EPS = 2.0 ** -25
QCONST = 4.0
SIM_SCALE = 0.25          # host scales t_norm by this before casting

LAST_RESULTS = None    # per-core output dicts of the most recent run


def build_nc(dtype=DTYPE, kl=KL, with_index=True, reps=1):
    """Build the single-core Bass program (SPMD across 8 cores).

    with_index=False (v3): drop the max_index pass and cand_i output --
    the host recovers indices by recomputing the <=8 winning 500-wide
    chunks per row (candidate slot -> chunk is static). Halves DVE work.

    reps>1 wraps the whole per-core computation in a hardware For_i loop
    executing it `reps` times back-to-back (identical work each rep, same
    outputs rewritten). Used ONLY for timing: wall(reps=R2)-wall(reps=R1)
    isolates the on-device execution time per kernel run from the ~70 ms
    axon-tunnel dispatch latency that dominates a single staged call.
    """
    groups = [KT] * (kl // KT) if not with_index else groups_for(kl)
    n_grp = len(groups)
    ncand = 8 * n_grp
    mx = max(groups)
    # Bacc (not raw Bass): its compile() passes split multi-semaphore waits
    # (move_matmul_waits_to_ldweights / generate_event_semaphores) that the
    # walrus codegen's 1-wait-per-instruction limit requires.
    nc = bacc.Bacc()
    bankT = nc.declare_dram_parameter("bankT", [C, kl], dtype, isOutput=False)
    tT = nc.declare_dram_parameter("tT", [C, B], dtype, isOutput=False)
    cand_v = nc.declare_dram_parameter(
        "cand_v", [B, ncand], mybir.dt.float32, isOutput=True
    )
    cand_i = None
    if with_index:
        cand_i = nc.declare_dram_parameter(
            "cand_i", [B, ncand], mybir.dt.uint32, isOutput=True
        )

    bankT_r = bankT.rearrange("(c p) k -> p c k", p=128)  # [128, 4, kl]
    tT_r = tT.rearrange("(c p) b -> p c b", p=128)        # [128, 4, B]

    with tile.TileContext(nc) as tc:
        with (
            tc.tile_pool(name="const", bufs=1) as constp,
            # bufs=4: with the max_index pass gone the PE chain paces the
            # schedule, and 4-deep bank prefetch keeps it fed (model:
            # 67.5us vs 70.5us at bufs=3; saturates at 4).
            tc.tile_pool(name="bank", bufs=4) as bankp,
            tc.tile_pool(name="sim", bufs=2) as simp,
            tc.tile_pool(name="cand", bufs=1) as candp,
            tc.tile_pool(name="ps", bufs=8, space="PSUM") as psp,
        ):
            def body():
                tw = constp.tile([128, 4, B], dtype, tag="tw")
                nc.sync.dma_start(tw[:], tT_r[:])

                vals = [
                    candp.tile([128, n_grp, 8], mybir.dt.float32, tag=f"v{b}", name=f"vals{b}")
                    for b in range(2)
                ]
                idxs = None
                if with_index:
                    idxs = [
                        candp.tile([128, n_grp, 8], mybir.dt.uint32, tag=f"i{b}", name=f"idxs{b}")
                        for b in range(2)
                    ]

                kt = 0
                for g, chunk in enumerate(groups):
                    sims = [
                        simp.tile([128, mx], mybir.dt.float32, tag=f"s{b}", name=f"sim{b}")
                        for b in range(2)
                    ]
                    for j in range(chunk // KT):
                        bk = bankp.tile([128, 4, KT], dtype, tag="bank")
                        if kt == 0:
                            # split the first load per c-chunk so the first
                            # matmul starts after 1/4 of the transfer
                            # (model: 64.7us vs 67.5us)
                            for c in range(4):
                                nc.sync.dma_start(
                                    bk[:, c, :], bankT_r[:, c, 0:KT]
                                )
                        else:
                            nc.sync.dma_start(
                                bk[:], bankT_r[:, :, kt * KT:(kt + 1) * KT]
                            )
                        for b in range(2):
                            ps = psp.tile([128, KT], mybir.dt.float32, tag="ps")
                            for c in range(4):
                                nc.tensor.matmul(
                                    ps[:],
                                    tw[:, c, b * 128:(b + 1) * 128],
                                    bk[:, c, :],
                                    start=(c == 0),
                                    stop=(c == 3),
                                )
                            nc.scalar.copy(sims[b][:, j * KT:(j + 1) * KT], ps[:])
                        kt += 1
                    for b in range(2):
                        nc.vector.max(vals[b][:, g, :], sims[b][:, 0:chunk])
                        if with_index:
                            nc.vector.max_index(
                                idxs[b][:, g, :], vals[b][:, g, :], sims[b][:, 0:chunk]
                            )

                for b in range(2):
                    nc.sync.dma_start(cand_v[b * 128:(b + 1) * 128, :], vals[b][:])
                    if with_index:
                        nc.sync.dma_start(cand_i[b * 128:(b + 1) * 128, :], idxs[b][:])

            if reps == 1:
                body()
            else:
                with tc.For_i(0, reps, 1):
                    body()

    return nc


def _make_consts():
    """Host-side constant rows for the v2 tag matmuls, bf16 [1, 3500].

    Layout: [0:128) ones (rank-1 stationary); [500:1000) +4.0;
    [1000:1500) -4.0; [1500+j*500 : 2000+j*500) tag row for kt%4 == j:
    id*2^-25 with id = ((j*500+n) // SUB) % 16. All exact in bf16.
    """
    c = np.zeros((1, 3500), np.float32)
    c[0, 0:128] = 1.0
    c[0, 500:1000] = QCONST
    c[0, 1000:1500] = -QCONST
    n = np.arange(KT)
    for j in range(4):
        ids = (j * KT + n) // SUB % N_SUB
        c[0, 1500 + j * 500:2000 + j * 500] = ids * TAG_EPS
    return c.astype(ml_dtypes.bfloat16)


def build_nc_v2(dtype=mybir.dt.bfloat16, kl=KL):
    """Tagged single-scan variant: one DVE max8 pass, no max_index."""
    assert dtype == mybir.dt.bfloat16
    n_grp = kl // CHUNK
    ncand = 8 * n_grp
    nc = bacc.Bacc()
    bankT = nc.declare_dram_parameter("bankT", [C, kl], dtype, isOutput=False)
    tT = nc.declare_dram_parameter("tT", [C, B], dtype, isOutput=False)
    consts = nc.declare_dram_parameter("consts", [1, 3500], dtype, isOutput=False)
    cand_v = nc.declare_dram_parameter(
        "cand_v", [B, ncand], mybir.dt.float32, isOutput=True
    )

    bankT_r = bankT.rearrange("(c p) k -> p c k", p=128)  # [128, 4, kl]
    tT_r = tT.rearrange("(c p) b -> p c b", p=128)        # [128, 4, B]

    with tile.TileContext(nc) as tc:
        with (
            tc.tile_pool(name="const", bufs=1) as constp,
            tc.tile_pool(name="bank", bufs=3) as bankp,
            tc.tile_pool(name="sim", bufs=2) as simp,
            tc.tile_pool(name="cand", bufs=1) as candp,
            tc.tile_pool(name="ps", bufs=8, space="PSUM") as psp,
        ):
            tw = constp.tile([128, 4, B], dtype)
            nc.sync.dma_start(tw[:], tT_r[:])
            cst = constp.tile([1, 3500], dtype)
            nc.sync.dma_start(cst[:], consts[:])
            ones_r = cst[0:1, 0:128]
            q_r = cst[0:1, 500:1000]
            nq_r = cst[0:1, 1000:1500]
            tag_r = [cst[0:1, 1500 + j * 500:2000 + j * 500] for j in range(4)]

            vals = [
                candp.tile([128, n_grp, 8], mybir.dt.float32,
                           tag=f"v{b}", name=f"vals{b}")
                for b in range(2)
            ]

            for g in range(n_grp):
                sims = [
                    simp.tile([128, CHUNK], mybir.dt.float32,
                              tag=f"s{b}", name=f"sim{b}")
                    for b in range(2)
                ]
                for j in range(GRP):
                    kt = g * GRP + j
                    bk = bankp.tile([128, 4, KT], dtype, tag="bank")
                    nc.sync.dma_start(
                        bk[:], bankT_r[:, :, kt * KT:(kt + 1) * KT]
                    )
                    for b in range(2):
                        ps = psp.tile([128, KT], mybir.dt.float32, tag="ps",
                                      name="ps")
                        for c in range(4):
                            nc.tensor.matmul(
                                ps[:],
                                tw[:, c, b * 128:(b + 1) * 128],
                                bk[:, c, :],
                                start=(c == 0), stop=False,
                            )
                        # quantize then tag: +4, -4, +id*2^-25 (in order)
                        nc.tensor.matmul(ps[:], ones_r, q_r,
                                         start=False, stop=False)
                        nc.tensor.matmul(ps[:], ones_r, nq_r,
                                         start=False, stop=False)
                        nc.tensor.matmul(ps[:], ones_r, tag_r[j % 4],
                                         start=False, stop=True)
                        nc.scalar.copy(sims[b][:, j * KT:(j + 1) * KT], ps[:])
                for b in range(2):
                    nc.vector.max(vals[b][:, g, :], sims[b][:])

            for b in range(2):
                nc.sync.dma_start(cand_v[b * 128:(b + 1) * 128, :], vals[b][:])

    return nc


# ---------------------------------------------------------------------------
# v5: fp8 DoubleRow + direct-PSUM max8.
#
#   - bank/t cast to fp8e4 (TRN E4M3, max +-240) scaled by 16 on the host
#     (components ~N(0, 1/512); x16 keeps them in e4m3's normal range).
#     Sims come out scaled by 256 -- irrelevant for ranking.
#   - DoubleRow perf mode contracts 256 c-elements per matmul (2 fp8 values
#     per lane per cycle): 2 matmuls per 500-wide k-tile per b-half instead
#     of 4 bf16 ones. PE time roughly halves.
#   - PSUM pair-tiles [128, 2, 512] (two banks); each 500-wide matmul group
#     accumulates into its own bank; one DVE max8 reads both halves straight
#     from PSUM (strided AP, free size 1000). No ScalarE eviction, no SBUF
#     sim tiles. DVE cost: (120 + 1000) cyc @0.96 GHz ~= 1.17 us x 16 chunks.
#   - Device gives top-8 VALUES per 1000-wide chunk (no indices). Host picks
#     the top W5 chunks per row by chunk max (a chunk containing a true
#     top-5 element has max >= the 5th-best sim, so in exact arithmetic the
#     top-5 chunks by max cover all of them; W5 > 5 absorbs fp8 noise) and
#     recomputes those windows in exact fp32 -- final top-5 indices match the
#     fp32 reference exactly whenever coverage holds.
# ---------------------------------------------------------------------------
V5_KT = 500          # matmul tile (PSUM bank limit: 512 fp32)
V5_CHUNK = 1000      # max8 window: one PSUM pair-tile
V5_DMA_KT = 1000     # DMA tile width (1000B runs per partition, >= 512B line rate)
V5_W = 12            # windows recomputed per row on the host
V5_SCALE = 16.0
F8 = mybir.dt.float8e4


def build_nc_v5(kl=KL, reps=1):
    n_grp = kl // V5_CHUNK           # 16 chunks of 1000 per core
    ncand = 8 * n_grp                # 128 candidates per row per core
    nc = bacc.Bacc()
    bankT = nc.declare_dram_parameter("bankT", [C, kl], F8, isOutput=False)
    tT = nc.declare_dram_parameter("tT", [C, B], F8, isOutput=False)
    cand_v = nc.declare_dram_parameter(
        "cand_v", [B, ncand], mybir.dt.float32, isOutput=True
    )

    bankT_r = bankT.rearrange("(c p) k -> p c k", p=128)  # [128, 4, kl]
    tT_r = tT.rearrange("(c p) b -> p c b", p=128)        # [128, 4, B]

    with tile.TileContext(nc) as tc:
        with (
            tc.tile_pool(name="const", bufs=1) as constp,
            tc.tile_pool(name="bank", bufs=4) as bankp,
            tc.tile_pool(name="cand", bufs=1) as candp,
            tc.tile_pool(name="ps", bufs=4, space="PSUM") as psp,
        ):
            def body():
                tw = constp.tile([128, 4, B], F8, tag="tw")
                nc.sync.dma_start(tw[:], tT_r[:])

                vals = [
                    candp.tile([128, n_grp, 8], mybir.dt.float32,
                               tag=f"v{b}", name=f"vals{b}")
                    for b in range(2)
                ]

                for g in range(n_grp):
                    bk = bankp.tile([128, 4, V5_DMA_KT], F8, tag="bank")
                    if g == 0:
                        # split the first load per c-pair so the first
                        # matmul starts sooner
                        for c2 in range(2):
                            nc.sync.dma_start(
                                bk[:, c2 * 2:(c2 + 1) * 2, :],
                                bankT_r[:, c2 * 2:(c2 + 1) * 2, 0:V5_DMA_KT],
                            )
                    else:
                        nc.sync.dma_start(
                            bk[:], bankT_r[:, :, g * V5_DMA_KT:(g + 1) * V5_DMA_KT]
                        )
                    for b in range(2):
                        ps = psp.tile([128, 2, 512], mybir.dt.float32, tag="ps")
                        for j in range(2):          # two 500-wide halves
                            for c2 in range(2):     # two DoubleRow c-pairs
                                nc.tensor.matmul(
                                    ps[:, j, 0:V5_KT],
                                    tw[:, c2 * 2:(c2 + 1) * 2, b * 128:(b + 1) * 128],
                                    bk[:, c2 * 2:(c2 + 1) * 2, j * V5_KT:(j + 1) * V5_KT],
                                    start=(c2 == 0),
                                    stop=(c2 == 1),
                                    perf_mode=mybir.MatmulPerfMode.DoubleRow,
                                )
                        nc.vector.max(vals[b][:, g, :], ps[:, :, 0:V5_KT])

                for b in range(2):
                    nc.sync.dma_start(cand_v[b * 128:(b + 1) * 128, :], vals[b][:])

            if reps == 1:
                body()
            else:
                with tc.For_i(0, reps, 1):
                    body()

    return nc


# "v1": two DVE scans per chunk (max8 + max_index) -- simplest, and the
#       faster schedule under the TRN2 instruction cost model (87.8us vs
#       109.6us predicted per core; DVE-bound).
# "v2": tagged single-scan -- one DVE max8 pass; the PE quantizes sims
#       in-PSUM (+4/-4 rank-1s) and adds a sub-quantum subchunk tag that
#       the host decodes, trading DVE time for PE time. Better if real
#       silicon streams bf16 matmuls near the documented 131ns/MM rate.
# "v3": v1's matmul+max8 pipeline with NO max_index pass at all -- the
#       candidate slot already identifies the 500-wide chunk, so the host
#       recomputes the <=8 best chunks per row (~1 GFLOP) to recover exact
#       indices. Halves DVE work; model-predicted 70.5us vs 84.5us (v1).
# All validated on the fixed inputs (HW): v1 loss rel err 4.9e-5,
# v2 5.3e-6, v3 4.9e-5; purity exact in all.
MODE = "v5"

_NC_CACHE = {}


def _get_nc():
    key = (MODE, DTYPE)
    if key not in _NC_CACHE:
        if MODE == "v2":
            nc = build_nc_v2()
        elif MODE == "v3":
            nc = build_nc(DTYPE, with_index=False)
        elif MODE == "v5":
            nc = build_nc_v5()
        else:
            nc = build_nc(DTYPE)
        nc.finalize()
        _NC_CACHE[key] = nc
    return _NC_CACHE[key]


class _SpmdExec:
    """Cached jitted shard_map over the bass_exec custom call.

    Mirrors bass2jax.run_bass_via_pjrt's multi-core path but builds the
    jitted executable once, so repeated calls skip retrace/recompile.
    """

    def __init__(self, nc):
        bass2jax.install_neuronx_cc_hook()
        part_name = (
            nc.partition_id_tensor.name if nc.partition_id_tensor else None
        )
        in_names, out_names, out_avals = [], [], []
        for alloc in nc.m.functions[0].allocations:
            if not isinstance(alloc, mybir.MemoryLocationSet):
                continue
            name = alloc.memorylocations[0].name
            if alloc.kind == "ExternalInput":
                if name != part_name:
                    in_names.append(name)
            elif alloc.kind == "ExternalOutput":
                out_names.append(name)
                out_avals.append(
                    jax.core.ShapedArray(
                        tuple(alloc.tensor_shape), mybir.dt.np(alloc.dtype)
                    )
                )
        self.in_names = list(in_names)
        self.out_names = out_names
        self.out_avals = out_avals
        n_params = len(in_names)
        n_outs = len(out_names)
        bind_names = in_names + out_names
        if part_name is not None:
            bind_names = bind_names + [part_name]
        bind_names = tuple(bind_names)

        def _body(*args):
            operands = list(args)
            if part_name is not None:
                operands.append(bass2jax.partition_id_tensor())
            outs = bass2jax._bass_exec_p.bind(
                *operands,
                out_avals=tuple(out_avals),
                in_names=bind_names,
                out_names=tuple(out_names),
                lowering_input_output_aliases=(),
                sim_require_finite=True,
                sim_require_nnan=True,
                nc=nc,
            )
            return tuple(outs)

        devices = jax.devices()[:N_CORES]
        self.mesh = Mesh(np.asarray(devices), ("core",))
        in_specs = (PartitionSpec("core"),) * (n_params + n_outs)
        out_specs = (PartitionSpec("core"),) * n_outs
        self.fn = jax.jit(
            shard_map(
                _body,
                mesh=self.mesh,
                in_specs=in_specs,
                out_specs=out_specs,
                check_rep=False,
            ),
            donate_argnums=tuple(range(n_params, n_params + n_outs)),
            keep_unused=True,
        )

    def zero_outs(self):
        return [
            np.zeros((N_CORES * a.shape[0], *a.shape[1:]), a.dtype)
            for a in self.out_avals
        ]

    def __call__(self, concat_inputs):
        """concat_inputs: list matching in_names, each (N_CORES*dim0, ...)."""
        out_arrs = self.fn(*concat_inputs, *self.zero_outs())
        return [
            {
                name: np.asarray(out_arrs[i]).reshape(
                    N_CORES, *self.out_avals[i].shape
                )[c]
                for i, name in enumerate(self.out_names)
            }
            for c in range(N_CORES)
        ]


def build_timing_nc(reps):
    """The current MODE's device program with an in-NEFF For_i repeat loop.

    Used by test.py's slope timing: wall(R2)-wall(R1) over (R2-R1) reps
    isolates per-execution device time from the ~70-100 ms axon-tunnel
    dispatch latency. Each rep re-runs the full per-core computation
    (bank re-streamed from DRAM, outputs rewritten)."""
    if MODE == "v5":
        return build_nc_v5(reps=reps)
    if MODE == "v3":
        return build_nc(DTYPE, with_index=False, reps=reps)
    raise ValueError(f"timing builder not wired for MODE={MODE}")


_EXEC_CACHE = {}


def _get_exec():
    key = (MODE, DTYPE)
    if key not in _EXEC_CACHE:
        _EXEC_CACHE[key] = _SpmdExec(_get_nc())
    return _EXEC_CACHE[key]


def _np_dtype(dtype):
    return ml_dtypes.bfloat16 if dtype == mybir.dt.bfloat16 else np.float32


def _run_v1(exe, bank_sh, t, tT):
    """max8 + max_index path: returns per-row global top-5 indices."""
    global LAST_RESULTS
    np_dt = _np_dtype(DTYPE)
    tT_c = tT.astype(np_dt)
    concat = {
        "bankT": bank_sh,
        "tT": np.concatenate([tT_c] * N_CORES, axis=0),
    }
    results = exe([concat[n] for n in exe.in_names])
    LAST_RESULTS = results

    vals = np.stack([r["cand_v"] for r in results], axis=1)
    idx_l = np.stack(
        [r["cand_i"].astype(np.int64) for r in results], axis=1
    )
    groups = groups_for(KL)
    gbase = np.concatenate([[0], np.cumsum(groups)[:-1]]).astype(np.int64)
    base = (
        np.arange(N_CORES, dtype=np.int64)[None, :, None] * KL
        + np.repeat(gbase, 8)[None, None, :]
    )
    gidx = (idx_l + base).reshape(B, -1)            # global indices
    vals = vals.reshape(B, -1)                      # raw sim_t

    # Emulate the reference's comparison domain: fp32 dist_t with per-row
    # 1/||t_b|| folded back in; ties break toward the lowest global index.
    inv_t = 1.0 / np.maximum(np.linalg.norm(t, axis=1), EPS)   # [B]
    dist32 = (2.0 - 2.0 * vals * inv_t[:, None]).astype(np.float32)
    top5 = np.empty((B, TOPK), np.int64)
    for b in range(B):
        order = np.lexsort((gidx[b], dist32[b]))
        top5[b] = gidx[b][order[:TOPK]]
    return top5


N_WINDOWS = 10  # per-row candidate windows recomputed exactly on the host


def _run_v2(exe, bank_sh, t, bank):
    """Tagged single-scan path: returns per-row global top-5 indices."""
    global LAST_RESULTS
    bf = ml_dtypes.bfloat16
    t_n = t / np.maximum(np.linalg.norm(t, axis=1, keepdims=True), EPS)
    tw = np.ascontiguousarray((t_n * SIM_SCALE).T).astype(bf)   # [C, B]
    consts = _make_consts()
    concat = {
        "bankT": bank_sh,
        "tT": np.concatenate([tw] * N_CORES, axis=0),
        "consts": np.concatenate([consts] * N_CORES, axis=0),
    }
    results = exe([concat[n] for n in exe.in_names])
    LAST_RESULTS = results

    # packed candidates [B, N_CORES, NCAND]
    packed = np.stack([r["cand_v"] for r in results], axis=1)
    pk = packed.reshape(B, -1).astype(np.float64)    # [B, 512]
    # packed = q(sim) + id*2^-25 with q a multiple of 2^-21 (positive sims)
    y = np.round(pk / TAG_EPS).astype(np.int64)      # exact integer
    dec_id = np.mod(y, N_SUB)
    qsim = pk - dec_id * TAG_EPS                     # quantized scaled sim
    # window start (global bank row) per candidate
    cores = np.repeat(np.arange(N_CORES, dtype=np.int64), NCAND)[None, :]
    groups = np.tile(
        np.repeat(np.arange(N_GRP, dtype=np.int64), 8), N_CORES
    )[None, :]
    wstart = cores * KL + groups * CHUNK + dec_id * SUB   # [B, 512]

    # top-N_WINDOWS candidates per row by qsim; recompute those 125-wide
    # windows exactly (fp32 over the bf16-cast operands, matching the
    # device's computation up to summation order) and take the exact top-5.
    order = np.argsort(-qsim, axis=1, kind="stable")[:, :N_WINDOWS]
    sel_start = np.take_along_axis(wstart, order, axis=1)     # [B, W]

    bank_bf = bank.astype(bf).astype(np.float32)              # [K, C]
    t_bf = (t_n * SIM_SCALE).astype(bf).astype(np.float32)    # [B, C]
    flat_idx = (sel_start[:, :, None] +
                np.arange(SUB, dtype=np.int64)[None, None, :])  # [B, W, SUB]
    rows = bank_bf[flat_idx.reshape(-1)].reshape(B, N_WINDOWS * SUB, C)
    wsims = np.einsum("bkc,bc->bk", rows, t_bf)               # [B, W*SUB]
    widx = flat_idx.reshape(B, -1)                            # [B, W*SUB]

    top5 = np.empty((B, TOPK), np.int64)
    for b in range(B):
        # windows may overlap -> dedupe indices, keep exact values
        o = np.lexsort((widx[b], -wsims[b]))
        seen, picks = set(), []
        for i in o:
            gi = widx[b, i]
            if gi in seen:
                continue
            seen.add(gi)
            picks.append(gi)
            if len(picks) == TOPK:
                break
        top5[b] = picks
    return top5


def _run_v3(exe, bank_sh, t, bank):
    """Index-free path: per-chunk top-8 values only (exact fp32, a
    deterministic superset of the per-chunk top-5); the host recovers
    indices by recomputing the <=8 best 500-wide chunks per row."""
    global LAST_RESULTS
    np_dt = _np_dtype(DTYPE)
    tT_c = np.ascontiguousarray(t.T).astype(np_dt)
    concat = {
        "bankT": bank_sh,
        "tT": np.concatenate([tT_c] * N_CORES, axis=0),
    }
    results = exe([concat[n] for n in exe.in_names])
    LAST_RESULTS = results

    n_grp = KL // KT                                 # 32 chunks of 500
    vals = np.stack([r["cand_v"] for r in results], axis=1)
    vals = vals.reshape(B, -1)                       # [B, 8*32*8=2048]
    # candidate slot -> global chunk start (chunk known from position)
    cores = np.repeat(np.arange(N_CORES, dtype=np.int64), 8 * n_grp)
    chunks = np.tile(np.repeat(np.arange(n_grp, dtype=np.int64), 8), N_CORES)
    wstart = (cores * KL + chunks * KT)[None, :]     # [1, 2048]

    # every true top-5 element is a candidate with a top-5 value, so the
    # top-8 candidate windows per row cover them deterministically
    order = np.argsort(-vals, axis=1, kind="stable")[:, :8]
    sel = np.take_along_axis(np.broadcast_to(wstart, vals.shape),
                             order, axis=1)          # [B, 8]

    bf = ml_dtypes.bfloat16
    bank_bf = bank.astype(bf).astype(np.float32)     # [K, C]
    t_bf = t.astype(bf).astype(np.float32)           # [B, C]
    top5 = np.empty((B, TOPK), np.int64)
    span = np.arange(KT, dtype=np.int64)
    for b in range(B):
        starts = np.unique(sel[b])
        widx = (starts[:, None] + span[None, :]).reshape(-1)
        wsims = bank_bf[widx] @ t_bf[b]              # exact bf16-input sims
        o = np.lexsort((widx, -wsims))
        top5[b] = widx[o[:TOPK]]
    return top5


def _run_v5(exe, bank8_sh, t_n, bank):
    """fp8 path: per-chunk top-8 values (chunks of 1000); host selects the
    top V5_W chunks per row by chunk max and recomputes them in exact fp32."""
    global LAST_RESULTS
    f8 = ml_dtypes.float8_e4m3
    tw8 = np.ascontiguousarray((t_n * V5_SCALE).T).astype(f8)   # [C, B]
    concat = {
        "bankT": bank8_sh,
        "tT": np.concatenate([tw8] * N_CORES, axis=0),
    }
    results = exe([concat[n] for n in exe.in_names])
    LAST_RESULTS = results

    n_grp = KL // V5_CHUNK                            # 16 chunks per core
    vals = np.stack([r["cand_v"] for r in results], axis=1)
    vals = vals.reshape(B, N_CORES, n_grp, 8)
    cmax = vals[:, :, :, 0].reshape(B, -1)            # [B, 128] chunk maxes
    # top-W chunks per row by (approximate fp8) chunk max
    order = np.argsort(-cmax, axis=1, kind="stable")[:, :V5_W]   # [B, W]

    # group rows by selected window so each window's 1000x512 bank slice is
    # used as a zero-copy view in one BLAS call
    rows_for = [[] for _ in range(N_CORES * n_grp)]
    for b in range(B):
        for pos, w in enumerate(order[b]):
            rows_for[w].append((b, pos))

    wsims = np.full((B, V5_W, V5_CHUNK), -np.inf, np.float32)
    for w, entries in enumerate(rows_for):
        if not entries:
            continue
        core, g = divmod(w, n_grp)
        start = core * KL + g * V5_CHUNK
        block = bank[start:start + V5_CHUNK]          # view, no copy
        rows = [b for b, _ in entries]
        sims = block @ t_n[rows].T                    # [1000, len(rows)] fp32
        for i, (b, pos) in enumerate(entries):
            wsims[b, pos] = sims[:, i]

    wstart = np.take_along_axis(
        (np.arange(N_CORES * n_grp, dtype=np.int64) // n_grp * KL
         + np.arange(N_CORES * n_grp, dtype=np.int64) % n_grp * V5_CHUNK)[None, :]
        .repeat(B, axis=0),
        order, axis=1,
    )                                                 # [B, W] window starts
    widx = wstart[:, :, None] + np.arange(V5_CHUNK, dtype=np.int64)[None, None, :]
    widx = widx.reshape(B, -1)
    wsims = wsims.reshape(B, -1)

    top5 = np.empty((B, TOPK), np.int64)
    for b in range(B):
        o = np.lexsort((widx[b], -wsims[b]))
        top5[b] = widx[b][o[:TOPK]]
    return top5


def kernel(query, current_target, queue, labels, labels_queue):
    query = np.asarray(query, np.float32)
    t = np.asarray(current_target, np.float32)
    queue_f = np.asarray(queue, np.float32)
    labels = np.asarray(labels)
    labels_queue = np.asarray(labels_queue)

    # Host prep: normalize bank rows (fp32, matching reference), transpose.
    norms = np.maximum(np.linalg.norm(queue_f, axis=1), EPS)
    bank = queue_f / norms[:, None]                 # [K, C], normalized
    tT = np.ascontiguousarray(t.T)                  # [C, B]

    exe = _get_exec()
    # [8*C, KL]: core m's shard (rows m*C..(m+1)*C) is bank[m*KL:(m+1)*KL].T
    bankT_full = np.ascontiguousarray(
        bank.reshape(N_CORES, KL, C).transpose(0, 2, 1)
    )
    if MODE == "v5":
        t_n = t / np.maximum(np.linalg.norm(t, axis=1, keepdims=True), EPS)
        bank8_sh = (bankT_full * V5_SCALE).astype(
            ml_dtypes.float8_e4m3
        ).reshape(N_CORES * C, KL)
        top5 = _run_v5(exe, bank8_sh, t_n.astype(np.float32), bank)
    else:
        np_dt = _np_dtype(DTYPE)
        bank_sh = bankT_full.astype(np_dt).reshape(N_CORES * C, KL)
        if MODE == "v2":
            top5 = _run_v2(exe, bank_sh, t, bank)
        elif MODE == "v3":
            top5 = _run_v3(exe, bank_sh, t, bank)
        else:
            top5 = _run_v1(exe, bank_sh, t, tT)

    # dist_q at the selected indices + purity.
    q_norm = query / np.maximum(
        np.linalg.norm(query, axis=1, keepdims=True), EPS
    )
    rows = bank[top5.reshape(-1)].reshape(B, TOPK, C)          # normalized
    nn_dist_q = 2.0 - 2.0 * np.einsum(
        "bjc,bc->bj", rows.astype(np.float64), q_norm.astype(np.float64)
    )
    loss = nn_dist_q.mean()
    matches = labels_queue[top5] == labels[:, None]
    purity = matches.mean()
    return (np.float32(loss), np.float32(purity))

